# revision 42
# baseline (speedup 1.0000x reference)
"""BertSelfAttention (with segment-embedding score bias) on 8 trn2 NeuronCores.

Math implemented (reference semantics):
    q = X @ Wq.T + bq ; k = X @ Wk.T ; v = X @ Wv.T + bv      (per head h)
    scores = (q*s) @ k.T + (q + b_q_s) @ segrep.T + mask ;  s = 1/sqrt(DH)
    out = softmax(scores) @ v

Formulation: per head, augmented 128-deep contractions
    qhat = [q*s ; q + b_q_s]   (dims 0:64 scaled, 64:128 plain+bias)
    khat = [k   ; segrep     ] (segrep = seg_table[seg_ids] slice, host-prep)
    scores = qhat . khat  (exactly includes the segment term); mask is a
    per-key bias fused into the exp() activation. The K=128 contraction keeps
    the PE array fully occupied.
    Softmax denominator = ones-column appended to V in the PV matmul
    (ctx^T accumulated with V stationary, then transposed back per 128-query
    tile and scaled by the reciprocal denominator).

Sharding: tensor-parallel over heads; core c owns heads 2c, 2c+1.
Each core reads the full tokens, computes its head-slice and its slice of
the output; host concatenates along the hidden dim. No collectives.

Schedule (v2): a single slot-level software pipeline. 256 slots
(16 attention iterations x 16 key tiles). Each slot emits on the PE:
2 scores matmuls, up to 2 lagged PV units, ~2 projection matmuls from a
globally paced slab queue, and at most 2 transpose tasks. The exp() calls
(the Scalar engine is ~75% busy with them) are emitted one per slot so the
ACT engine is never starved: projections no longer run as multi-us bursts
that idle ACT, and all projection drains moved from ACT to the (idle) DVE.
PSUM budget (8 banks): scores 2x2 + ctx 2 + proj 1 + transpose 1.
"""

import os
import sys

for _p in ("/opt/trn_rl_repo", "/root/.axon_site/_ro/trn_rl_repo"):
    if os.path.isdir(_p) and _p not in sys.path:
        sys.path.append(_p)

import numpy as np
import ml_dtypes

B, S, NH, DH = 4, 2048, 16, 64
HID = NH * DH          # 1024
T = B * S              # 8192
N_CORES = 8
HPC = NH // N_CORES    # heads per core = 2
DPC = HPC * DH         # out dims per core = 128
SCALE = 1.0 / 8.0      # 1/sqrt(DH)
KT = HID // 128        # 8 contraction tiles
CHUNK = 1024           # token chunk for xb DMA
SKT = S // 128         # 16 key tiles per sequence
LAG = 3                # pv lags scores by LAG slots (ACT latency cover)

_cache = {}


def _build_program():
    import concourse.bacc as bacc
    import concourse.tile as tile
    from concourse import masks, mybir
    from contextlib import ExitStack

    bf16 = mybir.dt.bfloat16
    f32 = mybir.dt.float32
    Exp = mybir.ActivationFunctionType.Exp
    Mult = mybir.AluOpType.mult
    Add = mybir.AluOpType.add

    nc = bacc.Bacc("TRN2", target_bir_lowering=False, debug=False,
                   num_devices=N_CORES)
    xb = nc.dram_tensor("xb", [HID, T], bf16, kind="ExternalInput")
    wq = nc.dram_tensor("wq", [HID, DPC], bf16, kind="ExternalInput")
    wk = nc.dram_tensor("wk", [HID, DPC], bf16, kind="ExternalInput")
    wv = nc.dram_tensor("wv", [HID, DPC], bf16, kind="ExternalInput")
    srt = nc.dram_tensor("srt", [128, T], bf16, kind="ExternalInput")
    rb = nc.dram_tensor("rb", [128, 128], f32, kind="ExternalInput")
    bqa = nc.dram_tensor("bqa", [DPC, 1], f32, kind="ExternalInput")
    bqb = nc.dram_tensor("bqb", [DPC, 1], f32, kind="ExternalInput")
    bv = nc.dram_tensor("bv", [DPC, 1], f32, kind="ExternalInput")
    outd = nc.dram_tensor("out", [T, DPC], f32, kind="ExternalOutput")
    # cross-partition bounce for the plain-q half of qhat (per 512-tok slab)
    qbounce = nc.dram_tensor("qbounce", [2, T // 512, 64, 512], bf16)

    with tile.TileContext(nc) as tc, ExitStack() as octx:
        const = octx.enter_context(tc.tile_pool(name="const", bufs=1))
        res = octx.enter_context(tc.tile_pool(name="res", bufs=1))
        xt_pool = octx.enter_context(tc.tile_pool(name="xt", bufs=24))
        vt_pool = octx.enter_context(tc.tile_pool(name="vt", bufs=2))
        pt_pool = octx.enter_context(tc.tile_pool(name="pt", bufs=20))
        ctxs_pool = octx.enter_context(tc.tile_pool(name="ctxs", bufs=2))
        qstage_pool = octx.enter_context(tc.tile_pool(name="qstage", bufs=2))
        stage_pool = octx.enter_context(tc.tile_pool(name="stage", bufs=2))
        rcp_pool = octx.enter_context(tc.tile_pool(name="rcp", bufs=8))
        sc_psum = octx.enter_context(
            tc.tile_pool(name="scp", bufs=2, space="PSUM"))
        ctx_psum = octx.enter_context(
            tc.tile_pool(name="ctxp", bufs=1, space="PSUM"))
        proj_psum = octx.enter_context(
            tc.tile_pool(name="projp", bufs=1, space="PSUM"))
        tp_psum = octx.enter_context(
            tc.tile_pool(name="tpp", bufs=1, space="PSUM"))

        # ---------------- constants ----------------
        rb_sb = const.tile([128, 128], f32)
        bqa_sb = const.tile([DPC, 1], f32)
        bqb_sb = const.tile([DPC, 1], f32)
        bqb2_sb = const.tile([DPC, 1], f32)   # bqb with head halves swapped
        bv_sb = const.tile([DPC, 1], f32)
        ident = const.tile([128, 128], bf16)
        wq_sb = const.tile([128, KT, DPC], bf16)
        wk_sb = const.tile([128, KT, DPC], bf16)
        wv_sb = const.tile([128, KT, DPC], bf16)

        # persistent per-(batch, head) activations
        qhs, khs, vsbs = [], [], []
        for b in range(B):
            qhs.append([res.tile([128, S], bf16, tag=f"qh{b}{hl}",
                                 name=f"qh{b}{hl}") for hl in range(2)])
            khs.append([res.tile([128, S], bf16, tag=f"kh{b}{hl}",
                                 name=f"kh{b}{hl}") for hl in range(2)])
            v = res.tile([128, SKT * 130], bf16, tag=f"vsb{b}",
                         name=f"vsb{b}")
            vsbs.append(v)   # ones columns preset later (strided memset)

        # ---------------- DMA: token chunks ----------------
        xts = {}          # ci -> [8 xt tiles]

        def emit_chunk_dma(ci, eng=None, split=1, srt_too=True):
            eng = eng or nc.sync
            cs = slice(ci * CHUNK, (ci + 1) * CHUNK)
            tiles = []
            for kt in range(KT):
                xt = xt_pool.tile([128, CHUNK], bf16, tag="xt", name="xt")
                rs = slice(kt * 128, (kt + 1) * 128)
                if split == 1:
                    eng.dma_start(xt[:], xb[rs, cs])
                else:
                    h = CHUNK // split
                    for p in range(split):
                        eng.dma_start(
                            xt[:, p * h:(p + 1) * h],
                            xb[rs, ci * CHUNK + p * h:ci * CHUNK + (p + 1) * h])
                tiles.append(xt)
            xts[ci] = tiles
            if srt_too:
                emit_srt_dma(ci, eng)

        def emit_srt_dma(ci, eng=None):
            eng = eng or nc.sync
            cs = slice(ci * CHUNK, (ci + 1) * CHUNK)
            b = ci // 2
            ls = slice((ci % 2) * CHUNK, (ci % 2) * CHUNK + CHUNK)
            eng.dma_start(khs[b][0][64:128, ls], srt[64:128, cs])
            eng.dma_start(khs[b][1][0:64, ls], srt[0:64, cs])

        # ---------------- projection slabs ----------------
        # slab = (kind, ci, half): 512 tokens, 8 accumulating matmuls into a
        # 1-bank psum tile, then a DVE drain.
        slab_done = set()

        class Slab:
            def __init__(self, kind, ci, half, ptag="proj", direct=False,
                         plain_tag=None):
                self.kind, self.ci, self.half = kind, ci, half
                self.ptag = ptag
                self.direct = direct          # q only: in-lane plain half
                self.plain_tag = plain_tag
                self.k = 0
                self.ps = None
                self.ps2 = None

            def _pool(self, tag):
                return {"sc": sc_psum, "ctx": ctx_psum, "proj": proj_psum,
                        "tp": tp_psum}[tag]

            def mm(self):
                if self.k == 0:
                    self.ps = self._pool(self.ptag).tile(
                        [128, 512], f32, tag=self.ptag,
                        name=f"ps_{self.kind}{self.ci}{self.half}")
                    if self.direct:
                        self.ps2 = self._pool(self.plain_tag).tile(
                            [128, 512], f32, tag=self.plain_tag,
                            name=f"ps2_{self.ci}{self.half}")
                w_sb = {"q": wq_sb, "k": wk_sb, "v": wv_sb}[self.kind]
                kt = self.k
                hs = slice(self.half * 512, self.half * 512 + 512)
                xsl = xts[self.ci][kt][:, hs]
                nc.tensor.matmul(self.ps[:], w_sb[:, kt, :], xsl,
                                 start=(kt == 0), stop=(kt == KT - 1))
                if self.direct:
                    # plain-q halves projected straight into the swapped
                    # partition ranges (M=64 column-tiled matmuls)
                    nc.tensor.matmul(self.ps2[64:128, :],
                                     wq_sb[:, kt, 0:64], xsl,
                                     start=(kt == 0), stop=(kt == KT - 1))
                    nc.tensor.matmul(self.ps2[0:64, :],
                                     wq_sb[:, kt, 64:128], xsl,
                                     start=(kt == 0), stop=(kt == KT - 1))
                self.k += 1
                return self.k == KT

            def drain(self):
                b, ci, half = self.ci // 2, self.ci, self.half
                ls = slice((ci % 2) * CHUNK + half * 512,
                           (ci % 2) * CHUNK + half * 512 + 512)
                si = ci * 2 + half
                if self.kind == "q":
                    # scaled halves: (q + bq) * s  == q*s + bqa  (bqa = bq*s)
                    nc.vector.tensor_scalar(
                        out=qhs[b][0][0:64, ls], in0=self.ps[0:64, :],
                        scalar1=SCALE, scalar2=bqa_sb[0:64, 0:1],
                        op0=Mult, op1=Add)
                    nc.vector.tensor_scalar(
                        out=qhs[b][1][64:128, ls], in0=self.ps[64:128, :],
                        scalar1=SCALE, scalar2=bqa_sb[64:128, 0:1],
                        op0=Mult, op1=Add)
                    if self.direct:
                        # plain halves already in the right partitions
                        nc.vector.tensor_scalar(
                            out=qhs[b][0][64:128, ls],
                            in0=self.ps2[64:128, :],
                            scalar1=bqb2_sb[64:128, 0:1], scalar2=None,
                            op0=Add)
                        nc.vector.tensor_scalar(
                            out=qhs[b][1][0:64, ls], in0=self.ps2[0:64, :],
                            scalar1=bqb2_sb[0:64, 0:1], scalar2=None,
                            op0=Add)
                        slab_done.add((self.kind, ci, half))
                        return
                    # plain half: q + (bq + b_q_s), bounced cross-partition
                    qs = qstage_pool.tile([128, 512], bf16, tag="qstage",
                                          name="qs")
                    nc.vector.tensor_scalar(
                        out=qs[:], in0=self.ps[:], scalar1=bqb_sb[:, 0:1],
                        scalar2=None, op0=Add)
                    nsp = 2 if ci <= 1 else 1   # split b0 bounces (latency)
                    hw = 512 // nsp
                    for p in range(nsp):
                        ws = slice(p * hw, (p + 1) * hw)
                        nc.sync.dma_start(qbounce[0, si, :, ws], qs[0:64, ws])
                        nc.sync.dma_start(qbounce[1, si, :, ws],
                                          qs[64:128, ws])
                    for p in range(nsp):
                        ws = slice(p * hw, (p + 1) * hw)
                        nc.sync.dma_start(
                            qhs[b][0][64:128, ls][:, ws], qbounce[0, si, :, ws])
                        nc.sync.dma_start(
                            qhs[b][1][0:64, ls][:, ws], qbounce[1, si, :, ws])
                elif self.kind == "k":
                    nc.vector.tensor_copy(khs[b][0][0:64, ls],
                                          self.ps[0:64, :])
                    nc.vector.tensor_copy(khs[b][1][64:128, ls],
                                          self.ps[64:128, :])
                else:  # v
                    vt = vt_pool.tile([128, 512], bf16, tag="vt", name="vt")
                    nc.vector.tensor_scalar(
                        out=vt[:], in0=self.ps[:], scalar1=bv_sb[:, 0:1],
                        scalar2=None, op0=Add)
                    for tt in range(4):
                        gt = (ci % 2) * 8 + half * 4 + tt
                        tp_queue.append(("v", b, gt, vt, tt,
                                         cur_slot[0] + 2))
                slab_done.add((self.kind, ci, half))

        # ---------------- transpose task machinery ------------------------
        # Phase 1 (PE): transpose into a slice of an 8-deep psum ring.
        # Phase 2 (DVE, deferred >= 1 slot): copy / rcp+scale consuming the
        # slice. The deferral keeps the in-order DVE queue from blocking on
        # a PE transpose that has not run yet (head-of-line coupling).
        tp_queue = []      # ("v", b, gt, vt, tt, min_slot)
                           # ("n", it, qt, ctxs, min_slot)
        dve_queue = []     # ("v2"/"n2", ..., slice, min_slot)
        tp_state = {"tile": None, "idx": 0}
        cur_slot = [0]

        def run_tp_task(task, s):
            sl = tp_state["tile"][:, tp_state["idx"] % 8, :]
            tp_state["idx"] += 1
            if task[0] == "v":
                _, b, gt, vt, tt, _ = task
                nc.tensor.transpose(sl, vt[:, tt * 128:(tt + 1) * 128],
                                    ident[:])
                dve_queue.append(("v2", b, gt, sl, s + 1))
            else:
                _, it, qt, ctxs, _ = task
                ctp = sl[:, 0:65]
                nc.tensor.transpose(ctp, ctxs[:, qt * 128:(qt + 1) * 128],
                                    ident[0:65, 0:65])
                dve_queue.append(("n2", it, qt, ctp, s + 1))

        def run_dve_task(task):
            if task[0] == "v2":
                _, b, gt, sl, _ = task
                nc.vector.tensor_copy(
                    vsbs[b][:, gt * 130:(gt + 1) * 130]
                    .rearrange("p (h x) -> p h x", h=2)[:, :, 0:64],
                    sl.rearrange("p (h d) -> p h d", h=2))
                vsb_ready[b].add(gt)
            else:
                _, it, qt, ctp, _ = task
                b, hl, qh = it // 4, (it // 2) % 2, it % 2
                gq = qh * 8 + qt
                rcp = rcp_pool.tile([128, 1], f32, tag="rcp", name="rcp")
                nc.vector.reciprocal(rcp[:], ctp[:, 64:65])
                nc.vector.tensor_scalar_mul(
                    get_stage(b)[:, gq * 128 + hl * 64:
                                 gq * 128 + hl * 64 + 64],
                    ctp[:, 0:64], rcp[:, 0:1])
                norm_left[it] -= 1
                if norm_left[it] == 0 and hl == 1:
                    flush_half(b, qh)

        vsb_ready = [set() for _ in range(B)]

        # ---------------- output staging ----------------
        stages = {}

        def get_stage(b):
            if b not in stages:
                stages[b] = stage_pool.tile([128, 16 * 128], f32,
                                            tag="stage", name=f"stage{b}")
            return stages[b]

        def flush_half(b, qh):
            nc.sync.dma_start(
                outd[b * S + qh * 1024:b * S + qh * 1024 + 1024, :]
                .rearrange("(gq q) hd -> q gq hd", q=128),
                get_stage(b)[:].rearrange("q (gq hd) -> q gq hd",
                                          hd=DPC)[:, qh * 8:(qh + 1) * 8, :])

        # ---------------- prologue ----------------
        # Parallel DMA issue: Sync carries Wk + chunk0 (split for multi-
        # engine bandwidth); the (idle) ACT engine carries Wq/Wv, chunk1,
        # chunk2, srt and the small consts.
        def emit_w_dma(w_sb, w, eng, pieces):
            src = w.rearrange("(kt p) d -> p kt d", p=128)
            step = KT // pieces
            for i in range(pieces):
                eng.dma_start(w_sb[:, i * step:(i + 1) * step, :],
                              src[:, i * step:(i + 1) * step, :])

        # DMA issue costs ~0.8us per instruction, serialized per issuing
        # engine — so the two startup-critical chunks are issued from TWO
        # engines in parallel (Sync: chunk0; ACT, idle until its first exp
        # at ~28us: chunk1), with everything else behind them.
        emit_w_dma(wk_sb, wk, nc.sync, 4)
        emit_chunk_dma(0, nc.sync, split=2, srt_too=False)
        nc.sync.dma_start(bqa_sb[:], bqa[:])
        nc.sync.dma_start(bqb_sb[:], bqb[:])
        emit_w_dma(wv_sb, wv, nc.sync, 2)
        nc.sync.dma_start(bv_sb[:], bv[:])
        emit_chunk_dma(2, nc.sync, split=1)
        emit_w_dma(wq_sb, wq, nc.scalar, 4)
        emit_srt_dma(0, nc.scalar)
        nc.scalar.dma_start(rb_sb[:], rb[:])
        nc.scalar.dma_start(bqb2_sb[64:128, 0:1], bqb[0:64, 0:1])
        nc.scalar.dma_start(bqb2_sb[0:64, 0:1], bqb[64:128, 0:1])
        emit_srt_dma(1, nc.scalar)
        emit_chunk_dma(1, nc.scalar, split=2, srt_too=False)

        masks.make_identity(nc, ident[:])
        # preload the exp table set on ACT while DMAs are in flight
        dum = rcp_pool.tile([128, 1], f32, tag="rcp", name="dum")
        nc.scalar.activation(dum[:], ident[:, 0:1], Exp, bias=0.0, scale=0.0)

        # PE warmup: dense matmuls un-throttle the HAM clock gate while the
        # first input DMAs are still in flight. Uses a 'proj' bank so the
        # 'sc' tiles that wave A needs are not serialized behind it.
        wup = proj_psum.tile([128, 512], f32, tag="proj", name="wup")
        for _ in range(70):
            nc.tensor.matmul(wup[:, 0:128], ident[:], ident[:],
                             start=True, stop=True)

        # wave A: k + both q slabs of chunk 0, interleaved by kt so the PE
        # follows the chunk-0 DMA as tiles land. The q slabs project their
        # plain halves directly (no DRAM bounce on the startup critical
        # path). 5 concurrent 1-bank psum tiles, borrowed across pools.
        waveA = [Slab("k", 0, 0, "sc"),
                 Slab("q", 0, 0, "sc", direct=True, plain_tag="ctx"),
                 Slab("q", 0, 1, "proj", direct=True, plain_tag="tp")]
        for kt in range(KT):
            for sl_ in waveA:
                sl_.mm()
        for sl_ in waveA:
            sl_.drain()
        # wave B: second half of chunk-0 keys
        kB = Slab("k", 0, 1, "sc")
        for kt in range(KT):
            kB.mm()
        kB.drain()

        # persistent 8-deep transpose psum ring (1 bank, 'tp' tag)
        tp_state["tile"] = tp_psum.tile([128, 8, 128], bf16, tag="tp",
                                        name="tptile")

        # preset the denominator ones-columns of vsb (strided, cheap)
        for b in range(B):
            nc.vector.memset(
                vsbs[b][:].rearrange("p (g x) -> p g x", x=65)[:, :, 64:65],
                1.0)

        # global proj slab queue (remaining work), ordered by deadline
        proj_queue = []
        for bb in range(B):
            c0, c1 = 2 * bb, 2 * bb + 1
            if bb == 0:
                order = [("v", c0, 0), ("v", c0, 1), ("k", c1, 0),
                         ("k", c1, 1), ("q", c1, 0), ("q", c1, 1),
                         ("v", c1, 0), ("v", c1, 1)]
            else:
                order = [("q", c0, 0), ("q", c0, 1), ("k", c0, 0),
                         ("k", c0, 1), ("v", c0, 0), ("v", c0, 1),
                         ("k", c1, 0), ("k", c1, 1), ("q", c1, 0),
                         ("q", c1, 1), ("v", c1, 0), ("v", c1, 1)]
            proj_queue.extend(Slab(k, c, h) for (k, c, h) in order)

        chunk_dma_at = {12: 3, 16: 4, 32: 5, 64: 6, 80: 7}

        # ---------------- main slot loop ----------------
        pts = {}            # (it, j) -> pt tile
        pv_queue = []       # (it, j) in emission order
        ctxps = {}          # it -> ctx psum tile
        norm_copy_slot = {}
        norm_left = {}
        proj_mms = 0
        proj_credit = 0.0
        cur = None          # current slab being emitted

        def emit_scores(it, kt, s):
            b, hl, qh = it // 4, (it // 2) % 2, it % 2
            # emission-order safety: operand slabs must already be emitted
            assert ("k", 2 * b + kt // 8, (kt % 8) // 4) in slab_done
            assert ("q", 2 * b + qh, 0) in slab_done
            assert ("q", 2 * b + qh, 1) in slab_done
            sp = sc_psum.tile([128, 1024], f32, tag="sc", name="sp")
            ksl = khs[b][hl][:, kt * 128:(kt + 1) * 128]
            for nn in range(2):
                qsl = qhs[b][hl][:, qh * 1024 + nn * 512:
                                 qh * 1024 + (nn + 1) * 512]
                nc.tensor.matmul(sp[:, nn * 512:(nn + 1) * 512], ksl, qsl,
                                 start=True, stop=True)
            pt = pt_pool.tile([128, 1024], bf16, tag="pt", name="pt")
            col = hl * 64 + b * 16 + kt
            nc.scalar.activation(pt[:], sp[:], Exp,
                                 bias=rb_sb[:, col:col + 1], scale=1.0)
            pts[(it, kt)] = pt
            pv_queue.append((it, kt))

        def pv_ready(it, j, s):
            b, hl, qh = it // 4, (it // 2) % 2, it % 2
            if s < it * 16 + j + LAG:
                return False
            if j not in vsb_ready[b]:
                return False
            if j == 0:
                if it > 0 and (it - 1 not in norm_copy_slot
                               or s < norm_copy_slot[it - 1] + 3):
                    return False
            return True

        def emit_pv(it, j, s):
            b, hl, qh = it // 4, (it // 2) % 2, it % 2
            if j == 0:
                ctxps[it] = ctx_psum.tile([65, 1024], f32, tag="ctx",
                                          name=f"ctxp{it}")
            ctxp = ctxps[it]
            vb = j * 130 + hl * 65
            pt = pts.pop((it, j))
            for nn in range(2):
                nc.tensor.matmul(ctxp[:, nn * 512:(nn + 1) * 512],
                                 vsbs[b][:, vb:vb + 65],
                                 pt[:, nn * 512:(nn + 1) * 512],
                                 start=(j == 0), stop=(j == SKT - 1))
            if j == SKT - 1:
                emit_norm_copy(it, s)

        def emit_norm_copy(it, s):
            ctxp = ctxps.pop(it)
            ctxs = ctxs_pool.tile([65, 1024], bf16, tag="ctxs", name="ctxs")
            nc.vector.tensor_copy(ctxs[:, 0:512], ctxp[:, 0:512])
            nc.vector.tensor_copy(ctxs[:, 512:1024], ctxp[:, 512:1024])
            norm_copy_slot[it] = s
            norm_left[it] = 8
            for qt in range(8):
                tp_queue.append(("n", it, qt, ctxs, s + 2))

        def pump_pv(s, budget=None):
            if budget is None:
                budget = 2 if len(pv_queue) > 6 else 1
            n = 0
            while n < budget and pv_queue:
                it, j = pv_queue[0]
                if not pv_ready(it, j, s):
                    break
                pv_queue.pop(0)
                emit_pv(it, j, s)
                n += 1

        def pump_proj(s):
            nonlocal proj_credit, proj_mms, cur
            proj_credit += 4.0 if proj_mms < 48 else 1.9
            while proj_credit >= 1.0:
                if cur is None:
                    if not proj_queue:
                        proj_credit = 0.0
                        return
                    cur = proj_queue.pop(0)
                done = cur.mm()
                proj_mms += 1
                proj_credit -= 1.0
                if done:
                    cur.drain()
                    cur = None

        def pump_tp(s):
            budget = 2 if len(tp_queue) > 6 else 1
            n = 0
            while n < budget and tp_queue:
                task = tp_queue[0]
                if task[-1] > s:
                    break
                tp_queue.pop(0)
                run_tp_task(task, s)
                n += 1

        def pump_dve(s):
            n = 0
            while n < 3 and dve_queue:
                task = dve_queue[0]
                if task[-1] > s:
                    break
                dve_queue.pop(0)
                run_dve_task(task)
                n += 1

        for s in range(256):
            it, kt = s // 16, s % 16
            cur_slot[0] = s
            if s in chunk_dma_at:
                emit_chunk_dma(chunk_dma_at[s])
            # pv/proj first: when scores(s) must wait on exp(s-2) freeing
            # its psum buffer, the in-order PE queue has already been handed
            # this slot's other work.
            pump_pv(s)
            pump_proj(s)
            emit_scores(it, kt, s)
            pump_tp(s)
            pump_dve(s)

        # ---------------- epilogue ----------------
        s = 256
        while pv_queue or tp_queue or dve_queue:
            cur_slot[0] = s
            pump_pv(s, budget=6)
            pump_tp(s)
            pump_dve(s)
            s += 1
            assert s < 400, "scheduler wedged"

    nc.compile()
    return nc


def get_program():
    if "nc" not in _cache:
        _cache["nc"] = _build_program()
    return _cache["nc"]


def make_in_maps(hidden_states, attention_mask, seg_ids, Wq, bq, Wk, Wv, bv,
                 seg_table, b_q_s):
    """Host-side shard + layout prep. Cheap (weights/bias reshapes, one bf16
    cast of X, 2-row segment gather); all O(T*S) math stays on device."""
    bf = ml_dtypes.bfloat16
    X = np.asarray(hidden_states, np.float32).reshape(T, HID)
    xb = np.ascontiguousarray(X.astype(bf).T)
    m = np.asarray(seg_ids).reshape(T).astype(np.int64)
    mask = np.asarray(attention_mask, np.float32).reshape(B, S)
    st = np.asarray(seg_table, np.float32)              # [2, HID]
    bqs = np.asarray(b_q_s, np.float32).reshape(NH, DH)
    Wq = np.asarray(Wq, np.float32)
    Wk = np.asarray(Wk, np.float32)
    Wv = np.asarray(Wv, np.float32)
    bq = np.asarray(bq, np.float32)
    bv = np.asarray(bv, np.float32)

    # mask-only per-key bias, same layout for both heads of a core:
    # rb[key, hl*64 + b*16 + kt] = mask[b, kt*128+key]
    rb_half = mask.reshape(B, 16, 128).transpose(2, 0, 1).reshape(128, 64)
    rb_c = np.ascontiguousarray(
        np.concatenate([rb_half, rb_half], axis=1).astype(np.float32))

    in_maps = []
    for c in range(N_CORES):
        h0, h1 = c * HPC, c * HPC + 1
        s0, s1 = slice(h0 * DH, (h0 + 1) * DH), slice(h1 * DH, (h1 + 1) * DH)
        # one plain Q weight slice; bias vectors: bqa = bq*s (scaled path
        # applies q*s + bq*s), bqb = bq + b_q_s (plain path)
        bqa_c = np.concatenate([bq[s0], bq[s1]]) * SCALE
        bqb_c = np.concatenate([bq[s0] + bqs[h0], bq[s1] + bqs[h1]])
        # segrep^T halves: [0:64]=head1, [64:128]=head0
        srt_c = np.empty((128, T), np.float32)
        srt_c[0:64, :] = st[np.ix_(m, range(s1.start, s1.stop))].T
        srt_c[64:128, :] = st[np.ix_(m, range(s0.start, s0.stop))].T
        sl = slice(c * DPC, (c + 1) * DPC)
        in_maps.append({
            "xb": xb,
            "wq": np.ascontiguousarray(Wq[sl, :].T).astype(bf),
            "wk": np.ascontiguousarray(Wk[sl, :].T).astype(bf),
            "wv": np.ascontiguousarray(Wv[sl, :].T).astype(bf),
            "srt": srt_c.astype(bf),
            "rb": rb_c,
            "bqa": np.ascontiguousarray(bqa_c.reshape(DPC, 1)),
            "bqb": np.ascontiguousarray(bqb_c.reshape(DPC, 1)),
            "bv": np.ascontiguousarray(bv[sl].reshape(DPC, 1)),
        })
    return in_maps


def assemble_output(results):
    return np.concatenate(
        [np.asarray(r["out"], np.float32).reshape(B, S, DPC) for r in results],
        axis=2)


def kernel(hidden_states, attention_mask, seg_ids, Wq, bq, Wk, Wv, bv,
           seg_table, b_q_s):
    from concourse.bass_utils import run_bass_kernel_spmd
    nc = get_program()
    in_maps = make_in_maps(hidden_states, attention_mask, seg_ids, Wq, bq,
                           Wk, Wv, bv, seg_table, b_q_s)
    res = run_bass_kernel_spmd(nc, in_maps, list(range(N_CORES)))
    return assemble_output(res.results)


if __name__ == "__main__":
    get_program()
    print("program built + compiled ok")
